# revision 33
# baseline (speedup 1.0000x reference)
"""Associative-embedding (AE) loss kernel for Trainium2, 8 NeuronCores.

Problem: tags [32, 262144, 1] f32, keypoints [32, 30, 17, 2] int
(col0 = flat heatmap index, col1 = valid flag). Output [32, 2] f32 =
stack([pull, push], axis=1) per batch.

Strategy (pure data parallel, 4 batches per core):
  - Host packs the VALID keypoints of the core's 4 batches densely into
    C = ceil(n_valid/128) slots of 128 partitions (C = 12 at ~70% valid
    density vs 17 for the naive person x joint layout). For each slot it
    emits an int32 flat offset into the tags shard and an assignment
    matrix A[c] in [128, 120] with A[c][slot, person] = valid/cnt[person].
  - Device: C indirect DMAs (standard DGE InstDMACopy, one index per
    partition each — the only gather primitive this runtime supports)
    fetch the tag values; per call the DVE squares the column and the PE
    accumulates A[c]^T @ [v, v^2] into PSUM -> per-person [mean, m2], plus
    a row-form v^T @ A[c] -> meanrow. All hidden under the next gather.
  - push: sq = (m2 - mean^2)/30 per person (DVE, from PSUM).
  - pull: exponent matrix built as Z = mean x meanrow (rank-1 matmul)
    + ones x (-msqrow/2) (rank-1 matmul) + const penalty (SBUF constant,
    one DVE add) where the penalty encodes the same-batch mask and the
    1/1740 pull scale in log space; exp(2*Z - msq[p]) runs on the scalar
    engine with per-partition bias and free-axis accumulation.
  - Final [128,4]^T @ [128,2] matmul (plus a rank-1 bias matmul removing
    the diagonal terms) reduces persons -> per-batch [pull, push].

Each core returns its own [4, 2] rows; the host concatenates to [32, 2].
"""

import os
import sys

import numpy as np

if "/opt/trn_rl_repo" not in sys.path:
    sys.path.insert(0, "/opt/trn_rl_repo")

import concourse.bacc as bacc
import concourse.bass as bass
import concourse.tile as tile
from concourse import mybir
from concourse.bass_utils import run_bass_kernel_spmd

# Problem constants (hardcoded per the harness contract)
B, N, D = 32, 262144, 1
P, J = 30, 17
NCORES = 8
BL = B // NCORES          # 4 local batches per core
NFLAT = BL * N            # 1048576 f32 elements in the per-core tags shard
PP = 128                  # slot partitions
NPER = BL * P             # 120 persons per core
PULL_SCALE = 0.5 / (P * (P - 1) / 2.0) * 0.5      # 1/1740
PEN_IN = -float(np.log(PULL_SCALE))               # ~7.46, same-batch offdiag
PEN_OUT = 60.0                                    # exp(-60) == 0 in f32

_F32 = mybir.dt.float32
_I32 = mybir.dt.int32


def _build_bass(C: int):
    nc = bacc.Bacc("TRN2", target_bir_lowering=False, debug=False,
                   num_devices=NCORES)

    tags_ext = nc.dram_tensor("tags", [NFLAT, 1], _F32, kind="ExternalInput")
    fidx_ext = nc.dram_tensor("fidx", [PP, C], _I32, kind="ExternalInput")
    a_ext = nc.dram_tensor("amat", [PP, C, NPER], _F32, kind="ExternalInput")
    ws_ext = nc.dram_tensor("wsel", [PP, BL], _F32, kind="ExternalInput")
    zc0_ext = nc.dram_tensor("zc0", [PP, PP], _F32, kind="ExternalInput")
    br_ext = nc.dram_tensor("brow", [1, 2], _F32, kind="ExternalInput")
    out_ext = nc.dram_tensor("out", [BL, 2], _F32, kind="ExternalOutput")

    with tile.TileContext(nc) as tc:
        with tc.tile_pool(name="sb", bufs=1) as pool, \
             tc.tile_pool(name="ps", bufs=1, space="PSUM") as psum:
            fidx_t = pool.tile([PP, C], _I32)
            nc.sync.dma_start(fidx_t[:], fidx_ext[:])

            # Warmup indirect DMA (constant offsets, scratch dest), placed at
            # the head of the Pool queue: absorbs the Q7 ucode fetch and
            # dynamic-DMA ring init while the fidx DMA is still in flight.
            with tc.high_priority():
                warm_off = pool.tile([PP, 1], _I32)
                nc.gpsimd.memset(warm_off[:], 0)
                warm_dst = pool.tile([PP, 1], _F32)
                nc.gpsimd.indirect_dma_start(
                    out=warm_dst[:], out_offset=None, in_=tags_ext[:],
                    in_offset=bass.IndirectOffsetOnAxis(
                        ap=warm_off[:], axis=0),
                )


            a_t = pool.tile([PP, C, NPER], _F32)
            nc.sync.dma_start(a_t[:], a_ext[:])
            ws_t = pool.tile([PP, BL], _F32)
            nc.sync.dma_start(ws_t[:], ws_ext[:])
            # Z-factor scratch + the constant penalty matrix (loaded early)
            ones1 = pool.tile([1, PP], _F32)
            nc.vector.memset(ones1[:], 1.0)
            mr1 = pool.tile([1, PP], _F32)
            nc.vector.memset(mr1[:], 0.0)
            z_ps = psum.tile([PP, PP], _F32)
            zc0_t = pool.tile([PP, PP], _F32)
            nc.sync.dma_start(zc0_t[:], zc0_ext[:])
            brow_t = pool.tile([1, 2], _F32)
            nc.sync.dma_start(brow_t[:], br_ext[:])
            mm_sb = pool.tile([PP, 2], _F32)      # [mean, m2] padded to 128
            nc.vector.memset(mm_sb[:], 0.0)

            # Packed gather + accumulate: psum[person, :] = sum_c A_c^T [v v^2]
            rhs_t = pool.tile([PP, C, 2], _F32)
            mm_ps = psum.tile([NPER, 2], _F32)
            mrow_ps = psum.tile([1, NPER], _F32)
            for c in range(C):
                nc.gpsimd.indirect_dma_start(
                    out=rhs_t[:, c, 0:1], out_offset=None, in_=tags_ext[:],
                    in_offset=bass.IndirectOffsetOnAxis(
                        ap=fidx_t[:, c:c + 1], axis=0),
                )
                nc.vector.tensor_scalar(
                    out=rhs_t[:, c, 1:2], in0=rhs_t[:, c, 0:1],
                    scalar1=rhs_t[:, c, 0:1], scalar2=None,
                    op0=mybir.AluOpType.mult,
                )
                nc.tensor.matmul(mrow_ps[:], rhs_t[:, c, 0:1], a_t[:, c, :],
                                 start=(c == 0), stop=(c == C - 1),
                                 skip_group_check=True)
                nc.tensor.matmul(mm_ps[:], a_t[:, c, :], rhs_t[:, c, :],
                                 start=(c == 0), stop=(c == C - 1),
                                 skip_group_check=True)

            # Pairwise pull (halved exponent, Exp scale=2):
            # Z[p,q] = mp mq - msq[q]/2 - PEN_OUT/2 + (D/2) same(p,q)
            # exp arg = 2*(Z - msq[p]/2); const part (zc0) added via DVE.
            nc.vector.tensor_copy(mr1[0:1, :NPER], mrow_ps[:])
            zc = pool.tile([1, PP], _F32)
            nc.vector.scalar_tensor_tensor(
                out=zc[:], in0=mr1[:], scalar=-0.5, in1=mr1[:],
                op0=mybir.AluOpType.mult, op1=mybir.AluOpType.mult,
            )
            nc.tensor.matmul(z_ps[:], mr1[:], mr1[:], start=True, stop=False,
                             skip_group_check=True)
            nc.tensor.matmul(z_ps[:], ones1[:], zc[:], start=False, stop=True,
                             skip_group_check=True)
            zfull = pool.tile([PP, PP], _F32)
            nc.vector.scalar_tensor_tensor(
                out=zfull[:], in0=z_ps[:], scalar=1.0, in1=zc0_t[:],
                op0=mybir.AluOpType.mult, op1=mybir.AluOpType.add,
            )

            nc.scalar.activation(mm_sb[:NPER, :], mm_ps[:],
                                 mybir.ActivationFunctionType.Copy)
            x_t = pool.tile([PP, 2], _F32)
            nc.vector.memset(x_t[:], 0.0)
            negmsq = pool.tile([PP, 1], _F32)
            nc.vector.tensor_scalar(
                out=negmsq[:], in0=mm_sb[:, 0:1], scalar1=mm_sb[:, 0:1],
                scalar2=-1.0,
                op0=mybir.AluOpType.mult, op1=mybir.AluOpType.mult,
            )
            # X[:,1] = per-person push term (m2 - mean^2)/P
            msq = pool.tile([PP, 1], _F32)
            nc.vector.tensor_scalar(
                out=msq[:], in0=negmsq[:], scalar1=-1.0, scalar2=None,
                op0=mybir.AluOpType.mult,
            )
            nc.vector.tensor_scalar(
                out=x_t[:NPER, 1:2], in0=mm_sb[:NPER, 1:2],
                scalar1=msq[:NPER], scalar2=1.0 / P,
                op0=mybir.AluOpType.subtract, op1=mybir.AluOpType.mult,
            )
            e_t = pool.tile([PP, PP], _F32)
            nc.scalar.activation(e_t[:], zfull[:],
                                 mybir.ActivationFunctionType.Exp, scale=2.0,
                                 bias=negmsq[:], accum_out=x_t[:, 0:1])

            # Persons -> batches: [pull, push]; the diagonal exp(-PEN_IN)
            # terms included in pull are removed by the brow bias matmul.
            out_ps = psum.tile([BL, 2], _F32)
            nc.tensor.matmul(out_ps[:], ws_t[:], x_t[:], start=True,
                             stop=False, skip_group_check=True)
            nc.tensor.matmul(out_ps[:], ones1[0:1, 0:BL], brow_t[:],
                             start=False, stop=True, skip_group_check=True)

            res = pool.tile([BL, 2], _F32)
            nc.scalar.activation(res[:], out_ps[:],
                                 mybir.ActivationFunctionType.Copy)
            nc.sync.dma_start(out_ext[:], res[:])

    nc.compile()
    return nc


def _prep_core_inputs(core: int, tags: np.ndarray, kp: np.ndarray,
                      C: int) -> dict:
    """Host-side preprocessing: shard + packed index/assignment tables."""
    b0 = core * BL
    t = np.ascontiguousarray(
        tags[b0:b0 + BL].reshape(NFLAT, 1).astype(np.float32, copy=False))

    idx = kp[b0:b0 + BL, :, :, 0].astype(np.int64)       # [BL,P,J]
    val = (kp[b0:b0 + BL, :, :, 1] == 1)                 # [BL,P,J]
    cnt = np.maximum(val.sum(-1), 1).astype(np.float32)  # [BL,P]

    flat = (idx + np.arange(BL)[:, None, None] * N)      # [BL,P,J] < NFLAT
    person = np.broadcast_to(
        np.arange(NPER).reshape(BL, P, 1), (BL, P, J))

    vflat = flat[val]          # [n_valid] gather offsets
    vperson = person[val]      # [n_valid] owning person
    wval = (1.0 / cnt.reshape(NPER))[vperson]            # weight 1/cnt
    n_valid = vflat.shape[0]
    assert n_valid <= C * PP

    fidx = np.zeros((PP, C), np.int32)
    amat = np.zeros((PP, C, NPER), np.float32)
    s = np.arange(n_valid)
    prt, call = s % PP, s // PP
    fidx[prt, call] = vflat
    amat[prt, call, vperson] = wval

    ws = np.zeros((PP, BL), np.float32)
    zc0 = np.full((PP, PP), -PEN_OUT / 2.0, np.float32)
    for b in range(BL):
        ws[b * P:(b + 1) * P, b] = 1.0
        zc0[b * P:(b + 1) * P, b * P:(b + 1) * P] += (PEN_OUT - PEN_IN) / 2.0

    brow = np.array([[-P * PULL_SCALE, 0.0]], np.float32)

    return {"tags": t, "fidx": fidx, "amat": amat, "wsel": ws,
            "zc0": zc0, "brow": brow}


_NC_CACHE = {}


def _get_nc(C: int):
    if C not in _NC_CACHE:
        _NC_CACHE[C] = _build_bass(C)
    return _NC_CACHE[C]


def _ensure_profile_hook():
    """Provide antenv.axon_hooks if the image's antenv lacks it, so
    run_bass_kernel_spmd(trace=True) can capture NTFF profiles under axon.
    Mirrors trn_agent_boot's ctypes shim over libaxon_pjrt.so."""
    try:
        from antenv.axon_hooks import get_axon_ntff_profile_hook  # noqa: F401
        return
    except ImportError:
        pass
    import contextlib
    import ctypes
    import types

    so_path = "/opt/axon/libaxon_pjrt.so"
    if not os.path.exists(so_path):
        return
    lib = ctypes.CDLL(so_path)
    if not hasattr(lib, "axon_start_nrt_profile"):
        return
    lib.axon_start_nrt_profile.argtypes = [ctypes.POINTER(ctypes.c_int64),
                                           ctypes.c_size_t]
    lib.axon_start_nrt_profile.restype = ctypes.c_int64
    lib.axon_stop_nrt_profile.argtypes = [ctypes.c_char_p]
    lib.axon_stop_nrt_profile.restype = ctypes.c_int64

    @contextlib.contextmanager
    def _hook(output_dir, device_ids):
        import jax
        jax.devices()
        if device_ids:
            ids = (ctypes.c_int64 * len(device_ids))(*device_ids)
            rc = lib.axon_start_nrt_profile(ids, len(device_ids))
        else:
            rc = lib.axon_start_nrt_profile(None, 0)
        if rc != 0:
            raise RuntimeError(f"axon_start_nrt_profile rc={rc}")
        try:
            yield
        finally:
            n = lib.axon_stop_nrt_profile(str(output_dir).encode())
            print(f"profile: {n} file(s) written to {output_dir}",
                  file=sys.stderr)

    mod = types.ModuleType("antenv.axon_hooks")
    _state = {"hook": _hook}
    mod.set_axon_ntff_profile_hook = lambda h: _state.__setitem__("hook", h)
    mod.get_axon_ntff_profile_hook = lambda: _state["hook"]
    sys.modules["antenv.axon_hooks"] = mod


def run(tags: np.ndarray, keypoints: np.ndarray, **spmd_kwargs):
    """Build in_maps, run on 8 cores, return ([32,2] f32, BassKernelResults)."""
    tags = np.asarray(tags)
    kp = np.asarray(keypoints)
    if spmd_kwargs.get("trace"):
        _ensure_profile_hook()
    val = (kp[..., 1] == 1).reshape(NCORES, -1)
    C = max(1, int(np.ceil(val.sum(axis=1).max() / PP)))
    nc = _get_nc(C)
    in_maps = [_prep_core_inputs(c, tags, kp, C) for c in range(NCORES)]
    results = run_bass_kernel_spmd(nc, in_maps, core_ids=list(range(NCORES)),
                                   **spmd_kwargs)
    out = np.concatenate([np.asarray(results.results[c]["out"])
                          for c in range(NCORES)], axis=0)
    return out.astype(np.float32), results


def kernel(tags: np.ndarray, keypoints: np.ndarray) -> np.ndarray:
    out, _ = run(tags, keypoints)
    return out
